# revision 1
# baseline (speedup 1.0000x reference)
import numpy as np
import jax
import jax.numpy as jnp
from functools import partial

# nn_LGGNet: B=64, N=62, D=4, T=512. Shard T across 8 cores (BN stats are
# per-timestep over (batch, feature), so T-sharding needs no cross-core comms).
B, N, D, T = 64, 62, 4, 512
NCORES = 8
EPS = 1e-5


def _bn(h, gamma, beta):
    mean = h.mean(axis=(1, 3), keepdims=True)
    var = h.var(axis=(1, 3), keepdims=True)
    return (h - mean) * jax.lax.rsqrt(var + EPS) * gamma[None, None, :, None] \
        + beta[None, None, :, None]


def _shard_fn(xt, local_w, local_b, global_adj, gcn_w, gcn_b,
              bn1_gamma, bn1_beta, bn2_gamma, bn2_beta):
    # xt: (T_loc, B, N, D)
    out = jax.nn.relu(xt * local_w[None, None] - local_b[None])
    s = jnp.einsum('tbnd,tbmd->tbnm', out, out)
    g = global_adj + global_adj.T
    adj = jax.nn.relu(s * g) + jnp.eye(N, dtype=xt.dtype)
    rowsum = adj.sum(-1)
    rowsum = jnp.where(rowsum == 0, 1.0, rowsum)
    d = rowsum ** -0.5
    adj = adj * d[..., :, None] * d[..., None, :]
    h = _bn(out, bn1_gamma, bn1_beta)
    h = h @ gcn_w - gcn_b[None]
    h = jax.nn.relu(jnp.einsum('tbnm,tbmd->tbnd', adj, h))
    h = _bn(h, bn2_gamma, bn2_beta)
    return h  # (T_loc, B, N, D)


_compiled = None


def _get_compiled():
    global _compiled
    if _compiled is None:
        devs = jax.devices()[:NCORES]
        fn = jax.pmap(_shard_fn, axis_name='i', devices=devs,
                      in_axes=(0, None, None, None, None, None,
                               None, None, None, None))
        _compiled = fn
    return _compiled


def kernel(x, local_w, local_b, global_adj, gcn_w, gcn_b,
           bn1_gamma, bn1_beta, bn2_gamma, bn2_beta):
    x = np.asarray(x, dtype=np.float32)
    # (B,N,D,T) -> (T,B,N,D) -> (8, T/8, B, N, D)
    xt = np.moveaxis(x, -1, 0)
    xt_sh = xt.reshape(NCORES, T // NCORES, B, N, D)
    fn = _get_compiled()
    h = fn(xt_sh, jnp.asarray(local_w), jnp.asarray(local_b),
           jnp.asarray(global_adj), jnp.asarray(gcn_w), jnp.asarray(gcn_b),
           jnp.asarray(bn1_gamma), jnp.asarray(bn1_beta),
           jnp.asarray(bn2_gamma), jnp.asarray(bn2_beta))
    h = np.asarray(h)                      # (8, T/8, B, N, D)
    h = h.reshape(T, B, N, D)
    return np.moveaxis(h, 0, -1).astype(np.float32)   # (B,N,D,T)
